# revision 17
# baseline (speedup 1.0000x reference)
"""MoE FFN (8 experts, top-2, SwiGLU) Trainium2 kernel.

Expert-parallel sharding: core e holds expert e's weights. The router runs on
the host (it already must, to decide dispatch): top-2 selection + softmax
combine weights are computed in numpy fp32, and each core receives its
gathered tokens plus a per-token combine weight. The device does only the
dense SwiGLU FFN:

    phase A:  hT[h, tok] = silu(wg.T @ x) * (wv.T @ x)      (bf16 matmuls)
    phase B:  y[tok, d]  = (hT.T @ wo) * w[tok]             (bf16 matmuls)

Everything heavyweight runs in bf16: rate-identical on the PE to fp32r but
half the HBM traffic (the ~24MB fp32 stream was gating the kernel's first
50us) and half the SBUF, letting all phase-A weights stay resident. Total
rms error ~0.5% -- well inside the 2e-2 gate.

Self-contained: shapes/sharding hardcoded for
x[2,2048,1024], 8 experts, d_expert=2048, top-2.
"""

import math
from contextlib import ExitStack

import ml_dtypes
import numpy as np

import concourse.bass as bass
import concourse.mybir as mybir
import concourse.tile as tile
from concourse import bacc
from concourse.bass_utils import run_bass_kernel_spmd
from neuron_dtypes import static_cast_fp32_to_fp32r

# ---- problem constants --------------------------------------------------
B, T, D = 2, 2048, 1024
N_TOK = B * T          # 4096 tokens
E = 8                  # experts == cores
H = 2048               # expert hidden dim
TOP_K = 2
P = 128

CAP = 1092             # per-expert token capacity (>= max load 1091, even chunks)
ND = D // P            # 8  d-tiles (contraction tiles of d_model)
NH = H // P            # 16 h-tiles
NT = math.ceil(CAP / P)  # 9 token tiles (8 full + 67)
NDC = D // 512         # 2  output column chunks

FP = mybir.dt.float32
FR = mybir.dt.float32r
BF = mybir.dt.bfloat16
AF = mybir.ActivationFunctionType
OP = mybir.AluOpType

# token chunks of CAP for phase A; >= 256 keeps fp32r at full PE rate and
# even widths satisfy the fp32r ISA restriction (moving/dst n_step even)
_A_CHUNKS = [(0, 512), (512, 324), (836, 256)]
assert sum(w for _, w in _A_CHUNKS) == CAP


def _round_fp32r(a):
    """Exact host-side fp32 -> fp32r rounding (bit layout stays fp32)."""
    return static_cast_fp32_to_fp32r(np.ascontiguousarray(a, dtype=np.float32)).view(
        np.float32
    )


def _emit(nc, tc, ctx, xtr_d, wgv_d, wo_d, wc_d, y_d):
    const = ctx.enter_context(tc.tile_pool(name="const", bufs=1))
    xc_pool = ctx.enter_context(tc.tile_pool(name="xc", bufs=1))
    ht_pool = ctx.enter_context(tc.tile_pool(name="ht", bufs=1))
    wgv_pool = ctx.enter_context(tc.tile_pool(name="wgv", bufs=16))
    wo_pool = ctx.enter_context(tc.tile_pool(name="wo", bufs=2))
    act_pool = ctx.enter_context(tc.tile_pool(name="act", bufs=3))
    yst_pool = ctx.enter_context(tc.tile_pool(name="yst", bufs=3))

    # ---- tiny resident constants ---------------------------------------
    wc_sb = const.tile([P, NT], FP)
    nc.scalar.dma_start(out=wc_sb[:], in_=wc_d.ap())
    junk = const.tile([P, 512], BF)
    nc.vector.memset(junk[:], 0.0)

    # ---- input streams (sync HWDGE ring; FIFO order = arrival priority) -
    xtr_ap = xtr_d.ap().rearrange("p (dt c) -> p dt c", dt=ND)
    wgv_ap = wgv_d.ap().rearrange("p (hk g dt q) -> p hk g dt q", hk=NH, g=2, q=P)
    wo_ap = wo_d.ap().rearrange("p (hk dc j) -> p hk dc j", hk=NH, dc=NDC)

    wg_sb = [None] * NH
    wv_sb = [None] * NH
    xc_sb = [None] * len(_A_CHUNKS)

    def load_w(hk, g):
        t = wgv_pool.tile(
            [P, ND, P], BF, tag=("wg" if g == 0 else "wv"),
            name=f"w{'gv'[g]}{hk}",
        )
        nc.sync.dma_start(out=t[:], in_=wgv_ap[:, hk, g])
        (wg_sb if g == 0 else wv_sb)[hk] = t

    def load_xc(ci, dlo, dhi):
        cs, cw = _A_CHUNKS[ci]
        if xc_sb[ci] is None:
            xc_sb[ci] = xc_pool.tile(
                [P, ND, cw], BF, tag=f"xc{ci}", name=f"xc{ci}"
            )
        nc.sync.dma_start(
            out=xc_sb[ci][:, dlo:dhi, :], in_=xtr_ap[:, dlo:dhi, cs:cs + cw]
        )

    # first chunk + first h-tile weights, finely interleaved so the first
    # phase-A chain starts as early as possible and stays DMA-paced
    load_w(0, 0)
    load_xc(0, 0, 4)
    load_w(0, 1)
    load_xc(0, 4, ND)
    for hk in (1, 2, 3):
        load_w(hk, 0)
        load_w(hk, 1)
    load_xc(1, 0, ND)
    load_xc(2, 0, ND)
    for hk in range(4, NH):
        load_w(hk, 0)
        load_w(hk, 1)
    wo_sb = []
    for dc in range(NDC):
        t = wo_pool.tile([P, NH, 512], BF, tag="wo")
        nc.sync.dma_start(out=t[:], in_=wo_ap[:, :, dc])
        wo_sb.append(t)

    with ExitStack() as fctx:
        # PSUM budget is exactly 8 banks: 6 single-buffered phase-A
        # accumulators (3 chunks x {g,v}) + 2 rotating phase-B banks (the
        # warm-up tile shares the phase-B tag).
        ps_a = fctx.enter_context(tc.tile_pool(name="psa", bufs=1, space="PSUM"))
        ps_y = fctx.enter_context(tc.tile_pool(name="psy", bufs=2, space="PSUM"))

        # PE warm-up: wide junk matmuls ramp the HAM clock gate and keep the
        # PE busy (no >3.4us idle window) until the first x/weight DMAs land.
        warm = ps_y.tile([P, 512], FP, name="warm", tag="py")
        for _ in range(11):
            nc.tensor.matmul(
                warm[:E, :], lhsT=junk[:, :E], rhs=junk[:],
                start=True, stop=True,
            )

        # ---- phase A: hT[h, tok] = silu(x@wg)^T * (x@wv)^T --------------
        # chunk-outer within each h-half: the first pass over a fresh chunk
        # touches the h-tiles in DMA-arrival order, so the PE never waits on
        # a weight tile that is behind other queued traffic. All three token
        # chunks accumulate in parallel banks so each stationary weight tile
        # is presented once per d-step, keeping the weight-load path off the
        # critical path for the narrow chunks.
        ht_sb = [
            ht_pool.tile([P, CAP], BF, tag=f"h{hk}", name=f"h{hk}")
            for hk in range(NH)
        ]
        NC_ = len(_A_CHUNKS)
        for half in range(2):
            first = half == 0
            for hk in range(half * 8, half * 8 + 8):
                pg = [
                    ps_a.tile([P, 512], FP, tag=f"pg{ci}", name=f"pg{ci}_{hk}")
                    for ci in range(NC_)
                ]
                pv = [
                    ps_a.tile([P, 512], FP, tag=f"pv{ci}", name=f"pv{ci}_{hk}")
                    for ci in range(NC_)
                ]
                # On the very first h-tiles the x chunks arrive one after
                # another; chunk-major chains there avoid stalling on xc1/xc2.
                order = (
                    [(ci, dn) for ci in range(NC_) for dn in range(ND)]
                    if first and hk < 2 else
                    [(ci, dn) for dn in range(ND) for ci in range(NC_)]
                )
                for w_sb, ps in ((wg_sb, pg), (wv_sb, pv)):
                    for ci, dn in order:
                        cw = _A_CHUNKS[ci][1]
                        nc.tensor.matmul(
                            ps[ci][:, :cw],
                            lhsT=w_sb[hk][:, dn, :],
                            rhs=xc_sb[ci][:, dn, :],
                            start=(dn == 0),
                            stop=(dn == ND - 1),
                        )
                for ci, (cs, cw) in enumerate(_A_CHUNKS):
                    # silu(g)*v = g*sigmoid(g)*v, decomposed (sim lacks Silu)
                    sg = act_pool.tile([P, 512], FP, tag="sg")
                    nc.scalar.activation(sg[:, :cw], pg[ci][:, :cw], AF.Sigmoid)
                    gs = act_pool.tile([P, 512], FP, tag="gs")
                    nc.vector.tensor_tensor(
                        gs[:, :cw], pg[ci][:, :cw], sg[:, :cw], op=OP.mult
                    )
                    nc.vector.tensor_tensor(
                        ht_sb[hk][:, cs:cs + cw], pv[ci][:, :cw], gs[:, :cw],
                        op=OP.mult,
                    )

        # ---- phase B: y[tok, d] = (hT^T @ wo) * w -----------------------
        for dc in range(NDC):
            dslice = slice(dc * 512, (dc + 1) * 512)
            for tt in range(NT):
                pt = min(P, CAP - tt * P)
                ts = slice(tt * P, tt * P + pt)
                py = ps_y.tile([P, 512], FP)
                for hk in range(NH):
                    nc.tensor.matmul(
                        py[:pt, :],
                        lhsT=ht_sb[hk][:, ts],
                        rhs=wo_sb[dc][:, hk, :],
                        start=(hk == 0),
                        stop=(hk == NH - 1),
                    )
                ysb = yst_pool.tile([P, 512], FP, tag="y")
                nc.vector.tensor_scalar(
                    ysb[:pt, :], py[:pt, :], wc_sb[:pt, tt:tt + 1], None,
                    op0=OP.mult,
                )
                nc.scalar.dma_start(out=y_d.ap()[ts, dslice], in_=ysb[:pt, :])


def _build():
    nc = bacc.Bacc("TRN2", target_bir_lowering=False, debug=False)
    xtr_d = nc.dram_tensor("xtr", [P, ND * CAP], BF, kind="ExternalInput")
    wgv_d = nc.dram_tensor("wgv", [P, NH * 2 * ND * P], BF, kind="ExternalInput")
    wo_d = nc.dram_tensor("wo", [P, NH * NDC * 512], BF, kind="ExternalInput")
    wc_d = nc.dram_tensor("wc", [P, NT], FP, kind="ExternalInput")
    y_d = nc.dram_tensor("y", [CAP, D], FP, kind="ExternalOutput")
    with tile.TileContext(nc) as tc:
        with ExitStack() as ctx:
            _emit(nc, tc, ctx, xtr_d, wgv_d, wo_d, wc_d, y_d)
    nc.compile()
    return nc


_NC = None


def _get_nc():
    global _NC
    if _NC is None:
        _NC = _build()
    return _NC


def _route(xf, gate_w, expert_bias):
    """Host-side replica of the reference router."""
    logits = xf @ gate_w + expert_bias          # [N, E] fp32
    m = logits.max(axis=-1, keepdims=True)
    p = np.exp(logits - m)
    p /= p.sum(axis=-1, keepdims=True)
    # ties -> lower index first, matching jax.lax.top_k
    order = np.argsort(-p, axis=-1, kind="stable")[:, :TOP_K]
    rw = np.take_along_axis(p, order, axis=-1)  # [N, K]
    rw = rw / (rw.sum(axis=-1, keepdims=True) + 1e-8)
    return order, rw


def _pack_wgv(wg_b, wv_b):
    """[D,H]x2 bf16 -> [P, NH*2*ND*P] matching the SBUF lhsT tile layout."""
    # target[p, hk, g, dn, q] = w_g[dn*P + p, hk*P + q]
    def tile4(w):
        return w.reshape(ND, P, NH, P).transpose(1, 2, 0, 3)  # [P, NH, ND, P]
    packed = np.stack([tile4(wg_b), tile4(wv_b)], axis=2)     # [P, NH, 2, ND, P]
    return np.ascontiguousarray(packed).reshape(P, -1)


def kernel(x, gate_w, expert_bias, w_gate, w_value, w_out, _trace=False):
    x = np.asarray(x, dtype=np.float32)
    gate_w = np.asarray(gate_w, dtype=np.float32)
    expert_bias = np.asarray(expert_bias, dtype=np.float32)
    w_gate = np.asarray(w_gate, dtype=np.float32)
    w_value = np.asarray(w_value, dtype=np.float32)
    w_out = np.asarray(w_out, dtype=np.float32)

    xf = np.ascontiguousarray(x.reshape(N_TOK, D))
    order, rw = _route(xf, gate_w, expert_bias)
    idx = [np.flatnonzero((order == e).any(axis=-1)) for e in range(E)]
    wtok = []
    for e in range(E):
        sel = (order[idx[e]] == e)
        wtok.append((rw[idx[e]] * sel).sum(axis=-1).astype(np.float32))
    n_rounds = max(1, math.ceil(max(len(i) for i in idx) / CAP))

    nc = _get_nc()
    wgv_e = [
        _pack_wgv(
            w_gate[e].astype(ml_dtypes.bfloat16),
            w_value[e].astype(ml_dtypes.bfloat16),
        )
        for e in range(E)
    ]
    # wo[p, hk, dc, j] = w_out[e][hk*P + p, dc*512 + j], bf16
    wo_e = [
        np.ascontiguousarray(
            w_out[e].astype(ml_dtypes.bfloat16).reshape(NH, P, NDC, 512)
            .transpose(1, 0, 2, 3)
        ).reshape(P, -1)
        for e in range(E)
    ]
    out = np.zeros((N_TOK, D), dtype=np.float32)
    last = None
    for r in range(n_rounds):
        in_maps = []
        for e in range(E):
            ids = idx[e][r * CAP:(r + 1) * CAP]
            ids_p = np.zeros(CAP, dtype=np.int64)
            ids_p[: len(ids)] = ids
            xt = xf[ids_p].T.astype(ml_dtypes.bfloat16)      # [D, CAP]
            xtr = np.ascontiguousarray(
                xt.reshape(ND, P, CAP).transpose(1, 0, 2)
            ).reshape(P, -1)
            w_pad = np.zeros(NT * P, dtype=np.float32)
            w_pad[: len(ids)] = wtok[e][r * CAP:(r + 1) * CAP]
            wc = np.ascontiguousarray(w_pad.reshape(NT, P).T)  # [P, NT]
            in_maps.append({
                "xtr": xtr,
                "wgv": wgv_e[e],
                "wo": wo_e[e],
                "wc": wc,
            })
        res = run_bass_kernel_spmd(
            nc, in_maps, core_ids=list(range(E)),
            trace=bool(_trace), trace_cores=list(range(E)) if _trace else None,
        )
        last = res
        for e in range(E):
            ids = idx[e][r * CAP:(r + 1) * CAP]
            if len(ids):
                out[ids] += res.results[e]["y"][: len(ids)]
    if _trace:
        kernel.last_results = last
    return out.reshape(B, T, D)
